# revision 10
# baseline (speedup 1.0000x reference)
"""CnnPatchDropout Trainium2 kernel.

Op: out[b,c,h,w] = mask[b, h//8, w//8] ? embed_img[c,h,w] : batch[b,c,h,w]
where mask is the fixed patch-dropout mask derived from jax.random.key(42)
(a constant of the problem — embedded below as packed bits) and
embed_img[c, ph*8+kh, pw*8+kw] = embed[ph*16+pw, c, kh, kw].

Strategy (pure data parallel, 8 images per core on 8 cores):
  - Host pre-transposes each core's batch shard to [b, h, c, w] so every
    DMA descriptor is a contiguous 32KB row (partition dim = h).
  - The mask depends only on (h, w), so it stays compact in SBUF
    ([h, b, w], 512KB) and is broadcast over the channel free-dim with a
    stride-0 access pattern feeding one copy_predicated per image on the
    vector engine (out = where(mask, embed, x), in place).
  - Loads on nc.sync (HWDGE ring 0), stores on nc.scalar (HWDGE ring 1),
    4-deep tile pool for load/compute/store overlap. The kernel is
    memory-bound: ~71MB of HBM traffic per core ≈ 200us roofline.
"""

import base64
import os

import numpy as np

B, C, H, W = 64, 64, 128, 128
N_CORES = 8
BPC = B // N_CORES  # images per core
PQ = 256  # patches per image (16x16 grid of 8x8 patches)
CHUNKS = int(os.environ.get("KERNEL_CHUNKS", "2"))  # c-splits per image
WORK_BUFS = int(os.environ.get("KERNEL_BUFS", "8"))

# np.packbits of the [64, 256] bool patch-dropout mask (reference.py
# _swap_mask(jax.random.key(42)); threefry is platform-deterministic).
_MASK_B64 = (
    "f1ZdZ4TF5bFnzLXXQjnEFTxeCkgnANSaUTRcxiSQrY8AAAAAAAAAAAAAAAAAAAAAAAAAAAAAAAAAAAAAAAAAAABA"
    "CxCBAAECACCAIACAQpIAAwAk4RiiAAYCQSCABSAAEESUBAAAEAQAAkhRkoIyImUBwEBCBIgAETIVggAiAHIAAAAA"
    "AAAAAAAAAAAAAAAAAAAAAAAAAAAAAAAAAAAAAAAAAAAAAAAAAAAAAAAAAAAAAAAAAAAAAAAAAAAAAAAAAAAAAAAA"
    "AAAAAAAAAAAAAAAAAAAAAAAAAAAAAAAAAAAwXgfLDUsdvSw7+sUMuh7JHVgZM3dCsbppFnIXPUFEeBRoUCCpuQX5"
    "QdmG6re8gEgOoDdKdy6bWl4KYYmI+yqCkNSAp8NeoaEJYcFELf4gRTJpAd+I8nGZrEHwzuTABTIAAAAAAAAAAAAA"
    "AAAAAAAAAAAAAAAAAAAAAAAAAAAAAAAAAAAAAAAAAAAAAAAAAAAAAAAAAAAAAAAAAAAAAAAAAQAZlbiUUNQGYA0g"
    "UI8FALLJCLUESArACEkETx8hIEiKqAMIARCAAoIigABICAAgQCMEAQgCACgAAQAAEAYjgAAAAAAAAAAAAAAAAAAA"
    "AAAAAAAAAAAAAAAAAAAAAAAAAAAAAAAAAAAAAAAAAAAAAAAAAAAAAAAAAAAAAAAAAAAEw2KI5TaPwshlGKFnAhUB"
    "RPAQ7sVg0zlYSlo+AOIHBQAAAAAAAAAAAAAAAAAAAAAAAAAAAAAAAAAAAAAAAAAAAAAKIFBAAAIFQjgaIJRAJZww"
    "iEAgDAIAACoUEABgCIgUgAMzMcCAAWQCCgKIIHhCAADKqDGgKhxSihgDixMBAAAAAAAAAAAAAAAAAAAAAAAAAAAA"
    "AAAAAAAAAAAAAAAAKgUIjEKCLkEUEwB/Ng42gVDMgRVUkEALM4KFiQaAGcIAAAAAAAAAAAAAAAAAAAAAAAAAAAAA"
    "AAAAAAAAAAAAAAAAAAAAAAAAAAAAAAAAAAAAAAAAAAAAAAAAAAAAAAAAAAACMCJRiCQggAAJKGEAAAQUcBgEABAA"
    "QAAAAAAgQQwIAAAAAAAACAIAEAACCAIADAGAAAQDAjhAwAAIJAABARzAgJ4g0aCwBMUpfoM/WGYY1D7QCFCgUood"
    "bIkMFv6ACAIAAFBACBwADIYKoCgkAiGApAQEBQUBAgBQCEQAUiAAAAAAAAAAAAAAAAAAAAAAAAAAAAAAAAAAAAAA"
    "AAAAAMmQAGa/k8emoXNCr0S2Q2Tm8ZDCjFoeMKfBo1/VK9KJAAAAAAAAAAAAAAAAAAAAAAAAAAAAAAAAAAAAAAAA"
    "AAAAAAAAAAAAAAAAAAAAAAAAAAAAAAAAAAAAAAAAAAAAACAEAAEIcQQAgCfQBICYUMAIkARasCkgxkAAAQEAAEBA"
    "AAAAAAAAAAAAAAAAAAAAAAAAAAAAAAAAAAAAAAAAAAAAAAAAAAAAAAAAAAAAAAAAAAAAAAAAAAAAAAAAAAAAAJwA"
    "mPhEexAcEyEDwTiyw8JPFDqKRIz/6+acQAgIWs6IBBAEgDVMBJEAAIGUAEeEAxgiEBUAFYIwADhAAAAkhg0AAAAA"
    "AAAAAAAAAAAAAAAAAAAAAAAAAAAAAAAAAAAAAAAAAAAAAAAAAAAAAAAAAAAAAAAAAAAAAAAAAAAAAAAAbnIPCR2g"
    "E5MmgguBBQBCGgYjkAyHDxQFkK0Q5ASZmAcIACAAQAAEQsAABiAAAEAIAAAAAAFAAgCAoEAiQSQAgAAAAAAAAAAA"
    "AAAAAAAAAAAAAAAAAAAAAAAAAAAAAAAADZA4IAALRju4H4hYSs6BjDEnKnTSnhEKsIJxxCKLzcFGEzyVpjoqkCap"
    "AICIJAAGKRKEEQwbAFKBwgEAAAAGIANktkCEwAQJhqBB5QEAwASAUEgAGQABWOIgNwABARCBAAAAAAAAAAAAAAAA"
    "AAAAAAAAAAAAAAAAAAAAAAAAAAAAAAAAAAAAAAAAAAAAAAAAAAAAAAAAAAAAAAAAAAAAAAAAAAAAAAAAAAAAAAAA"
    "AAAAAAAAAAAAAAAAAAAAAAAAUCAzDBRRIA9ElhhSBSIDAiiqIjEEyQMEAGAEtBrRmCAAAAAAAAAAAAAAAAAAAAAA"
    "AAAAAAAAAAAAAAAAAAAAAAAAAAAAAAAAAAAAAAAAAAAAAAAAAAAAAAAAAAAAAAAAAAAAAAAAAAAAAAAAAAAAAAAA"
    "AAAAAAAAAAAAAAAAAAAAAAAAAAAAAAAAAAAAAAAAAAAAAAAAAAAAAAAAAAAAAAAAAAAAAAAAAAAAAAAAAAAAAAAA"
    "AAAAAAAAAAAAAAAAAAAAAAAAAAAAAAAAAAAAAAAAAAAAAAAAAAAAAAAAAAADCUBEABMFIwNIFAEB4hBIBgcQlMCI"
    "CDCkYoAgKAIZawAAAAAAAAAAAAAAAAAAAAAAAAAAAAAAAAAAAAAAAAAAAAAAAAAAAAAAAAAAAAAAAAAAAAAAAAAA"
    "AAAAAAAAAAAAAAAAAAAAAAAAAAAAAAAAAAAAAAAAAAAAAAAAAAAAAIoAAAgKAUAIA2ACAQYEACaAIABACIlkCIAm"
    "IEAAAYQCAAAAAAAAAAAAAAAAAAAAAAAAAAAAAAAAAAAAAAAAAACKhfVjRv0Wj3OGlM+4SlPJoSR5ywVB+iZCpABY"
    "jC+n4gAAAAAAAAAAAAAAAAAAAAAAAAAAAAAAAAAAAAAAAAAAAAEBAlBBAAPaowCAMFABFGMt0AMAAEBmEEFA0gQj"
    "dBA="
)


def _mask_np() -> np.ndarray:
    """[B, PQ] bool dropout mask (True = patch replaced by embed)."""
    bits = np.unpackbits(np.frombuffer(base64.b64decode(_MASK_B64), np.uint8))
    return bits[: B * PQ].reshape(B, PQ).astype(bool)


_CACHE: dict = {}

# The BassKernelResults of the most recent kernel() call (exec_time_ns is
# populated when BASS_TRACE=1 is set in the environment).
LAST_RESULTS = None


def _build_nc():
    import concourse.bacc as bacc
    import concourse.mybir as mybir
    from concourse.tile import TileContext

    f32 = mybir.dt.float32
    # Bacc (not raw Bass): its finalize() splits multi-wait instructions into
    # the 1-wait-per-instruction form TRN2 codegen requires.
    nc = bacc.Bacc(None)
    x = nc.declare_dram_parameter("x", [BPC, H, C, W], f32, isOutput=False)
    e = nc.declare_dram_parameter("e", [H, C, W], f32, isOutput=False)
    m = nc.declare_dram_parameter("m", [H, BPC, W], mybir.dt.uint8, isOutput=False)
    y = nc.declare_dram_parameter("y", [BPC, H, C, W], f32, isOutput=True)

    with TileContext(nc) as tc:
        with (
            tc.tile_pool(name="const", bufs=1) as cpool,
            tc.tile_pool(name="work", bufs=WORK_BUFS) as pool,
        ):
            et = cpool.tile([H, C, W], f32)
            nc.sync.dma_start(out=et[:], in_=e[:])
            mt = cpool.tile([H, BPC, W], mybir.dt.uint8)
            nc.sync.dma_start(out=mt[:], in_=m[:])
            # Touch the constants on DVE so its vector clock observes both
            # DMA semaphores here; otherwise the first copy_predicated needs
            # 3 sync waits, which overflows the TT encoding's wait slots.
            warm = cpool.tile([H, 2], f32)
            nc.vector.tensor_copy(out=warm[:, 0:1], in_=et[:, 0, 0:1])
            nc.vector.tensor_copy(out=warm[:, 1:2], in_=mt[:, 0, 0:1])
            CH = C // CHUNKS  # channels per chunk
            for b in range(BPC):
                for k in range(CHUNKS):
                    cs = k * CH
                    t = pool.tile([H, CH, W], f32, tag="t")
                    nc.sync.dma_start(out=t[:], in_=x[b, :, cs : cs + CH, :])
                    mask_ap = mt[:, b : b + 1, :].broadcast_to([H, CH, W])
                    nc.vector.copy_predicated(t[:], mask_ap, et[:, cs : cs + CH, :])
                    nc.scalar.dma_start(out=y[b, :, cs : cs + CH, :], in_=t[:])
    # run the Bacc passes (register allocation, wait splitting) now —
    # run_bass_via_pjrt serializes the module without calling finalize().
    nc.finalize()
    return nc


def _prep_const_inputs():
    """embed-image [h, c, w] and per-core expanded masks [h, b, w]."""
    mask = _mask_np()
    m_img = mask.reshape(B, 16, 16)
    m_exp = np.repeat(np.repeat(m_img, 8, axis=1), 8, axis=2)  # [B, H, W] bool
    per_core_m = []
    for ci in range(N_CORES):
        mm = m_exp[ci * BPC : (ci + 1) * BPC].transpose(1, 0, 2)  # [H, BPC, W]
        per_core_m.append(np.ascontiguousarray(mm, dtype=np.uint8))
    return per_core_m


def kernel(batch: np.ndarray, embed: np.ndarray) -> np.ndarray:
    from concourse.bass_utils import run_bass_kernel_spmd

    global LAST_RESULTS
    batch = np.asarray(batch, dtype=np.float32)
    embed = np.asarray(embed, dtype=np.float32)

    if "nc" not in _CACHE:
        _CACHE["nc"] = _build_nc()
        _CACHE["masks"] = _prep_const_inputs()
    nc = _CACHE["nc"]
    per_core_m = _CACHE["masks"]

    # embed [PQ, C, 8, 8] -> embed-image [h, c, w]
    e_hcw = embed.reshape(16, 16, C, 8, 8).transpose(0, 3, 2, 1, 4).reshape(H, C, W)
    e_hcw = np.ascontiguousarray(e_hcw, dtype=np.float32)

    in_maps = []
    for ci in range(N_CORES):
        xb = batch[ci * BPC : (ci + 1) * BPC]  # [BPC, C, H, W]
        xt = np.ascontiguousarray(xb.transpose(0, 2, 1, 3))  # [BPC, H, C, W]
        in_maps.append({"x": xt, "e": e_hcw, "m": per_core_m[ci]})

    res = run_bass_kernel_spmd(nc, in_maps, core_ids=list(range(N_CORES)))
    LAST_RESULTS = res

    out = np.empty((B, C, H, W), dtype=np.float32)
    for ci in range(N_CORES):
        yt = res.results[ci]["y"]  # [BPC, H, C, W]
        out[ci * BPC : (ci + 1) * BPC] = yt.transpose(0, 2, 1, 3)
    return out


# revision 11
# speedup vs baseline: 1.0535x; 1.0535x over previous
"""CnnPatchDropout Trainium2 kernel.

Op: out[b,c,h,w] = mask[b, h//8, w//8] ? embed_img[c,h,w] : batch[b,c,h,w]
where mask is the fixed patch-dropout mask derived from jax.random.key(42)
(a constant of the problem — embedded below as packed bits) and
embed_img[c, ph*8+kh, pw*8+kw] = embed[ph*16+pw, c, kh, kw].

Strategy (pure data parallel, 8 images per core on 8 cores):
  - Host pre-transposes each core's batch shard to [b, h, c, w] so every
    DMA descriptor is a contiguous 32KB row (partition dim = h).
  - The mask depends only on (h, w), so it stays compact in SBUF
    ([h, b, w], 512KB) and is broadcast over the channel free-dim with a
    stride-0 access pattern feeding one copy_predicated per image on the
    vector engine (out = where(mask, embed, x), in place).
  - Loads on nc.sync (HWDGE ring 0), stores on nc.scalar (HWDGE ring 1),
    4-deep tile pool for load/compute/store overlap. The kernel is
    memory-bound: ~71MB of HBM traffic per core ≈ 200us roofline.
"""

import base64
import os

import numpy as np

B, C, H, W = 64, 64, 128, 128
N_CORES = 8
BPC = B // N_CORES  # images per core
PQ = 256  # patches per image (16x16 grid of 8x8 patches)
CHUNKS = int(os.environ.get("KERNEL_CHUNKS", "2"))  # c-splits per image
WORK_BUFS = int(os.environ.get("KERNEL_BUFS", "8"))

# np.packbits of the [64, 256] bool patch-dropout mask (reference.py
# _swap_mask(jax.random.key(42)); threefry is platform-deterministic).
_MASK_B64 = (
    "f1ZdZ4TF5bFnzLXXQjnEFTxeCkgnANSaUTRcxiSQrY8AAAAAAAAAAAAAAAAAAAAAAAAAAAAAAAAAAAAAAAAAAABA"
    "CxCBAAECACCAIACAQpIAAwAk4RiiAAYCQSCABSAAEESUBAAAEAQAAkhRkoIyImUBwEBCBIgAETIVggAiAHIAAAAA"
    "AAAAAAAAAAAAAAAAAAAAAAAAAAAAAAAAAAAAAAAAAAAAAAAAAAAAAAAAAAAAAAAAAAAAAAAAAAAAAAAAAAAAAAAA"
    "AAAAAAAAAAAAAAAAAAAAAAAAAAAAAAAAAAAwXgfLDUsdvSw7+sUMuh7JHVgZM3dCsbppFnIXPUFEeBRoUCCpuQX5"
    "QdmG6re8gEgOoDdKdy6bWl4KYYmI+yqCkNSAp8NeoaEJYcFELf4gRTJpAd+I8nGZrEHwzuTABTIAAAAAAAAAAAAA"
    "AAAAAAAAAAAAAAAAAAAAAAAAAAAAAAAAAAAAAAAAAAAAAAAAAAAAAAAAAAAAAAAAAAAAAAAAAQAZlbiUUNQGYA0g"
    "UI8FALLJCLUESArACEkETx8hIEiKqAMIARCAAoIigABICAAgQCMEAQgCACgAAQAAEAYjgAAAAAAAAAAAAAAAAAAA"
    "AAAAAAAAAAAAAAAAAAAAAAAAAAAAAAAAAAAAAAAAAAAAAAAAAAAAAAAAAAAAAAAAAAAEw2KI5TaPwshlGKFnAhUB"
    "RPAQ7sVg0zlYSlo+AOIHBQAAAAAAAAAAAAAAAAAAAAAAAAAAAAAAAAAAAAAAAAAAAAAKIFBAAAIFQjgaIJRAJZww"
    "iEAgDAIAACoUEABgCIgUgAMzMcCAAWQCCgKIIHhCAADKqDGgKhxSihgDixMBAAAAAAAAAAAAAAAAAAAAAAAAAAAA"
    "AAAAAAAAAAAAAAAAKgUIjEKCLkEUEwB/Ng42gVDMgRVUkEALM4KFiQaAGcIAAAAAAAAAAAAAAAAAAAAAAAAAAAAA"
    "AAAAAAAAAAAAAAAAAAAAAAAAAAAAAAAAAAAAAAAAAAAAAAAAAAAAAAAAAAACMCJRiCQggAAJKGEAAAQUcBgEABAA"
    "QAAAAAAgQQwIAAAAAAAACAIAEAACCAIADAGAAAQDAjhAwAAIJAABARzAgJ4g0aCwBMUpfoM/WGYY1D7QCFCgUood"
    "bIkMFv6ACAIAAFBACBwADIYKoCgkAiGApAQEBQUBAgBQCEQAUiAAAAAAAAAAAAAAAAAAAAAAAAAAAAAAAAAAAAAA"
    "AAAAAMmQAGa/k8emoXNCr0S2Q2Tm8ZDCjFoeMKfBo1/VK9KJAAAAAAAAAAAAAAAAAAAAAAAAAAAAAAAAAAAAAAAA"
    "AAAAAAAAAAAAAAAAAAAAAAAAAAAAAAAAAAAAAAAAAAAAACAEAAEIcQQAgCfQBICYUMAIkARasCkgxkAAAQEAAEBA"
    "AAAAAAAAAAAAAAAAAAAAAAAAAAAAAAAAAAAAAAAAAAAAAAAAAAAAAAAAAAAAAAAAAAAAAAAAAAAAAAAAAAAAAJwA"
    "mPhEexAcEyEDwTiyw8JPFDqKRIz/6+acQAgIWs6IBBAEgDVMBJEAAIGUAEeEAxgiEBUAFYIwADhAAAAkhg0AAAAA"
    "AAAAAAAAAAAAAAAAAAAAAAAAAAAAAAAAAAAAAAAAAAAAAAAAAAAAAAAAAAAAAAAAAAAAAAAAAAAAAAAAbnIPCR2g"
    "E5MmgguBBQBCGgYjkAyHDxQFkK0Q5ASZmAcIACAAQAAEQsAABiAAAEAIAAAAAAFAAgCAoEAiQSQAgAAAAAAAAAAA"
    "AAAAAAAAAAAAAAAAAAAAAAAAAAAAAAAADZA4IAALRju4H4hYSs6BjDEnKnTSnhEKsIJxxCKLzcFGEzyVpjoqkCap"
    "AICIJAAGKRKEEQwbAFKBwgEAAAAGIANktkCEwAQJhqBB5QEAwASAUEgAGQABWOIgNwABARCBAAAAAAAAAAAAAAAA"
    "AAAAAAAAAAAAAAAAAAAAAAAAAAAAAAAAAAAAAAAAAAAAAAAAAAAAAAAAAAAAAAAAAAAAAAAAAAAAAAAAAAAAAAAA"
    "AAAAAAAAAAAAAAAAAAAAAAAAUCAzDBRRIA9ElhhSBSIDAiiqIjEEyQMEAGAEtBrRmCAAAAAAAAAAAAAAAAAAAAAA"
    "AAAAAAAAAAAAAAAAAAAAAAAAAAAAAAAAAAAAAAAAAAAAAAAAAAAAAAAAAAAAAAAAAAAAAAAAAAAAAAAAAAAAAAAA"
    "AAAAAAAAAAAAAAAAAAAAAAAAAAAAAAAAAAAAAAAAAAAAAAAAAAAAAAAAAAAAAAAAAAAAAAAAAAAAAAAAAAAAAAAA"
    "AAAAAAAAAAAAAAAAAAAAAAAAAAAAAAAAAAAAAAAAAAAAAAAAAAAAAAAAAAADCUBEABMFIwNIFAEB4hBIBgcQlMCI"
    "CDCkYoAgKAIZawAAAAAAAAAAAAAAAAAAAAAAAAAAAAAAAAAAAAAAAAAAAAAAAAAAAAAAAAAAAAAAAAAAAAAAAAAA"
    "AAAAAAAAAAAAAAAAAAAAAAAAAAAAAAAAAAAAAAAAAAAAAAAAAAAAAIoAAAgKAUAIA2ACAQYEACaAIABACIlkCIAm"
    "IEAAAYQCAAAAAAAAAAAAAAAAAAAAAAAAAAAAAAAAAAAAAAAAAACKhfVjRv0Wj3OGlM+4SlPJoSR5ywVB+iZCpABY"
    "jC+n4gAAAAAAAAAAAAAAAAAAAAAAAAAAAAAAAAAAAAAAAAAAAAEBAlBBAAPaowCAMFABFGMt0AMAAEBmEEFA0gQj"
    "dBA="
)


def _mask_np() -> np.ndarray:
    """[B, PQ] bool dropout mask (True = patch replaced by embed)."""
    bits = np.unpackbits(np.frombuffer(base64.b64decode(_MASK_B64), np.uint8))
    return bits[: B * PQ].reshape(B, PQ).astype(bool)


_CACHE: dict = {}

# The BassKernelResults of the most recent kernel() call (exec_time_ns is
# populated when BASS_TRACE=1 is set in the environment).
LAST_RESULTS = None


def _build_nc():
    import concourse.bacc as bacc
    import concourse.mybir as mybir
    from concourse.tile import TileContext

    f32 = mybir.dt.float32
    # Bacc (not raw Bass): its finalize() splits multi-wait instructions into
    # the 1-wait-per-instruction form TRN2 codegen requires.
    nc = bacc.Bacc(None)
    x = nc.declare_dram_parameter("x", [BPC, H, C, W], f32, isOutput=False)
    e = nc.declare_dram_parameter("e", [H, C, W], f32, isOutput=False)
    m = nc.declare_dram_parameter("m", [H, BPC, W], mybir.dt.uint8, isOutput=False)
    y = nc.declare_dram_parameter("y", [BPC, H, C, W], f32, isOutput=True)

    with TileContext(nc) as tc:
        with (
            tc.tile_pool(name="const", bufs=1) as cpool,
            tc.tile_pool(name="work", bufs=WORK_BUFS) as pool,
        ):
            et = cpool.tile([H, C, W], f32)
            nc.sync.dma_start(out=et[:], in_=e[:])
            mt = cpool.tile([H, BPC, W], mybir.dt.uint8)
            nc.sync.dma_start(out=mt[:], in_=m[:])
            # Touch the constants on DVE so its vector clock observes both
            # DMA semaphores here; otherwise the first copy_predicated needs
            # 3 sync waits, which overflows the TT encoding's wait slots.
            warm = cpool.tile([H, 2], f32)
            nc.vector.tensor_copy(out=warm[:, 0:1], in_=et[:, 0, 0:1])
            nc.vector.tensor_copy(out=warm[:, 1:2], in_=mt[:, 0, 0:1])
            # chunk schedule: (image, c_start, c_len). Uniform C//CHUNKS
            # splits, except the last image tapers off in smaller chunks so
            # the serial tail (last load -> predicate -> last store) shrinks.
            CH = C // CHUNKS
            sched = []
            for b in range(BPC - 1):
                for k in range(CHUNKS):
                    sched.append((b, k * CH, CH))
            for k in range(CHUNKS - 1):
                sched.append((BPC - 1, k * CH, CH))
            cs = (CHUNKS - 1) * CH
            for step in (CH // 2, CH // 4, CH // 8, CH // 8):
                sched.append((BPC - 1, cs, step))
                cs += step
            assert cs == C
            for b, c0, cl in sched:
                t = pool.tile([H, CH, W], f32, tag="t")
                nc.sync.dma_start(out=t[:, :cl, :], in_=x[b, :, c0 : c0 + cl, :])
                mask_ap = mt[:, b : b + 1, :].broadcast_to([H, cl, W])
                nc.vector.copy_predicated(t[:, :cl, :], mask_ap, et[:, c0 : c0 + cl, :])
                nc.scalar.dma_start(out=y[b, :, c0 : c0 + cl, :], in_=t[:, :cl, :])
    # run the Bacc passes (register allocation, wait splitting) now —
    # run_bass_via_pjrt serializes the module without calling finalize().
    nc.finalize()
    return nc


def _prep_const_inputs():
    """embed-image [h, c, w] and per-core expanded masks [h, b, w]."""
    mask = _mask_np()
    m_img = mask.reshape(B, 16, 16)
    m_exp = np.repeat(np.repeat(m_img, 8, axis=1), 8, axis=2)  # [B, H, W] bool
    per_core_m = []
    for ci in range(N_CORES):
        mm = m_exp[ci * BPC : (ci + 1) * BPC].transpose(1, 0, 2)  # [H, BPC, W]
        per_core_m.append(np.ascontiguousarray(mm, dtype=np.uint8))
    return per_core_m


def kernel(batch: np.ndarray, embed: np.ndarray) -> np.ndarray:
    from concourse.bass_utils import run_bass_kernel_spmd

    global LAST_RESULTS
    batch = np.asarray(batch, dtype=np.float32)
    embed = np.asarray(embed, dtype=np.float32)

    if "nc" not in _CACHE:
        _CACHE["nc"] = _build_nc()
        _CACHE["masks"] = _prep_const_inputs()
    nc = _CACHE["nc"]
    per_core_m = _CACHE["masks"]

    # embed [PQ, C, 8, 8] -> embed-image [h, c, w]
    e_hcw = embed.reshape(16, 16, C, 8, 8).transpose(0, 3, 2, 1, 4).reshape(H, C, W)
    e_hcw = np.ascontiguousarray(e_hcw, dtype=np.float32)

    in_maps = []
    for ci in range(N_CORES):
        xb = batch[ci * BPC : (ci + 1) * BPC]  # [BPC, C, H, W]
        xt = np.ascontiguousarray(xb.transpose(0, 2, 1, 3))  # [BPC, H, C, W]
        in_maps.append({"x": xt, "e": e_hcw, "m": per_core_m[ci]})

    res = run_bass_kernel_spmd(nc, in_maps, core_ids=list(range(N_CORES)))
    LAST_RESULTS = res

    out = np.empty((B, C, H, W), dtype=np.float32)
    for ci in range(N_CORES):
        yt = res.results[ci]["y"]  # [BPC, H, C, W]
        out[ci * BPC : (ci + 1) * BPC] = yt.transpose(0, 2, 1, 3)
    return out


# revision 13
# speedup vs baseline: 1.2729x; 1.2082x over previous
"""CnnPatchDropout Trainium2 kernel.

Op: out[b,c,h,w] = mask[b, h//8, w//8] ? embed_img[c,h,w] : batch[b,c,h,w]
where mask is the fixed patch-dropout mask derived from jax.random.key(42)
(a constant of the problem — embedded below as packed bits) and
embed_img[c, ph*8+kh, pw*8+kw] = embed[ph*16+pw, c, kh, kw].

Strategy (pure data parallel, 8 images per core on 8 cores):
  - Host pre-transposes each core's batch shard to [b, h, c, w] so every
    DMA descriptor is a contiguous 32KB row (partition dim = h).
  - The mask depends only on (h, w), so it stays compact in SBUF
    ([h, b, w] uint8, 128KB) and is broadcast over the channel free-dim
    with a stride-0 access pattern feeding one copy_predicated per chunk
    on the vector engine (out = where(mask, embed, x), in place).
  - Images are processed in 32-channel chunks (2MB DMAs), loads on
    nc.sync (HWDGE ring 0), stores on nc.scalar (HWDGE ring 1), 8 work
    buffers for a deep load/compute/store pipeline; the last image
    tapers to smaller chunks to shrink the serial tail. The kernel is
    memory-bound: ~71MB of HBM traffic per core; measured ~179us/core
    (~400 GB/s effective, at the HBM roofline).
"""

import base64

import numpy as np

B, C, H, W = 64, 64, 128, 128
N_CORES = 8
BPC = B // N_CORES  # images per core
PQ = 256  # patches per image (16x16 grid of 8x8 patches)
CHUNKS = 2  # channel-splits per image (32-channel chunks, 2MB DMAs)
WORK_BUFS = 8  # work-tile slots: deep load/compute/store pipeline

# np.packbits of the [64, 256] bool patch-dropout mask (reference.py
# _swap_mask(jax.random.key(42)); threefry is platform-deterministic).
_MASK_B64 = (
    "f1ZdZ4TF5bFnzLXXQjnEFTxeCkgnANSaUTRcxiSQrY8AAAAAAAAAAAAAAAAAAAAAAAAAAAAAAAAAAAAAAAAAAABA"
    "CxCBAAECACCAIACAQpIAAwAk4RiiAAYCQSCABSAAEESUBAAAEAQAAkhRkoIyImUBwEBCBIgAETIVggAiAHIAAAAA"
    "AAAAAAAAAAAAAAAAAAAAAAAAAAAAAAAAAAAAAAAAAAAAAAAAAAAAAAAAAAAAAAAAAAAAAAAAAAAAAAAAAAAAAAAA"
    "AAAAAAAAAAAAAAAAAAAAAAAAAAAAAAAAAAAwXgfLDUsdvSw7+sUMuh7JHVgZM3dCsbppFnIXPUFEeBRoUCCpuQX5"
    "QdmG6re8gEgOoDdKdy6bWl4KYYmI+yqCkNSAp8NeoaEJYcFELf4gRTJpAd+I8nGZrEHwzuTABTIAAAAAAAAAAAAA"
    "AAAAAAAAAAAAAAAAAAAAAAAAAAAAAAAAAAAAAAAAAAAAAAAAAAAAAAAAAAAAAAAAAAAAAAAAAQAZlbiUUNQGYA0g"
    "UI8FALLJCLUESArACEkETx8hIEiKqAMIARCAAoIigABICAAgQCMEAQgCACgAAQAAEAYjgAAAAAAAAAAAAAAAAAAA"
    "AAAAAAAAAAAAAAAAAAAAAAAAAAAAAAAAAAAAAAAAAAAAAAAAAAAAAAAAAAAAAAAAAAAEw2KI5TaPwshlGKFnAhUB"
    "RPAQ7sVg0zlYSlo+AOIHBQAAAAAAAAAAAAAAAAAAAAAAAAAAAAAAAAAAAAAAAAAAAAAKIFBAAAIFQjgaIJRAJZww"
    "iEAgDAIAACoUEABgCIgUgAMzMcCAAWQCCgKIIHhCAADKqDGgKhxSihgDixMBAAAAAAAAAAAAAAAAAAAAAAAAAAAA"
    "AAAAAAAAAAAAAAAAKgUIjEKCLkEUEwB/Ng42gVDMgRVUkEALM4KFiQaAGcIAAAAAAAAAAAAAAAAAAAAAAAAAAAAA"
    "AAAAAAAAAAAAAAAAAAAAAAAAAAAAAAAAAAAAAAAAAAAAAAAAAAAAAAAAAAACMCJRiCQggAAJKGEAAAQUcBgEABAA"
    "QAAAAAAgQQwIAAAAAAAACAIAEAACCAIADAGAAAQDAjhAwAAIJAABARzAgJ4g0aCwBMUpfoM/WGYY1D7QCFCgUood"
    "bIkMFv6ACAIAAFBACBwADIYKoCgkAiGApAQEBQUBAgBQCEQAUiAAAAAAAAAAAAAAAAAAAAAAAAAAAAAAAAAAAAAA"
    "AAAAAMmQAGa/k8emoXNCr0S2Q2Tm8ZDCjFoeMKfBo1/VK9KJAAAAAAAAAAAAAAAAAAAAAAAAAAAAAAAAAAAAAAAA"
    "AAAAAAAAAAAAAAAAAAAAAAAAAAAAAAAAAAAAAAAAAAAAACAEAAEIcQQAgCfQBICYUMAIkARasCkgxkAAAQEAAEBA"
    "AAAAAAAAAAAAAAAAAAAAAAAAAAAAAAAAAAAAAAAAAAAAAAAAAAAAAAAAAAAAAAAAAAAAAAAAAAAAAAAAAAAAAJwA"
    "mPhEexAcEyEDwTiyw8JPFDqKRIz/6+acQAgIWs6IBBAEgDVMBJEAAIGUAEeEAxgiEBUAFYIwADhAAAAkhg0AAAAA"
    "AAAAAAAAAAAAAAAAAAAAAAAAAAAAAAAAAAAAAAAAAAAAAAAAAAAAAAAAAAAAAAAAAAAAAAAAAAAAAAAAbnIPCR2g"
    "E5MmgguBBQBCGgYjkAyHDxQFkK0Q5ASZmAcIACAAQAAEQsAABiAAAEAIAAAAAAFAAgCAoEAiQSQAgAAAAAAAAAAA"
    "AAAAAAAAAAAAAAAAAAAAAAAAAAAAAAAADZA4IAALRju4H4hYSs6BjDEnKnTSnhEKsIJxxCKLzcFGEzyVpjoqkCap"
    "AICIJAAGKRKEEQwbAFKBwgEAAAAGIANktkCEwAQJhqBB5QEAwASAUEgAGQABWOIgNwABARCBAAAAAAAAAAAAAAAA"
    "AAAAAAAAAAAAAAAAAAAAAAAAAAAAAAAAAAAAAAAAAAAAAAAAAAAAAAAAAAAAAAAAAAAAAAAAAAAAAAAAAAAAAAAA"
    "AAAAAAAAAAAAAAAAAAAAAAAAUCAzDBRRIA9ElhhSBSIDAiiqIjEEyQMEAGAEtBrRmCAAAAAAAAAAAAAAAAAAAAAA"
    "AAAAAAAAAAAAAAAAAAAAAAAAAAAAAAAAAAAAAAAAAAAAAAAAAAAAAAAAAAAAAAAAAAAAAAAAAAAAAAAAAAAAAAAA"
    "AAAAAAAAAAAAAAAAAAAAAAAAAAAAAAAAAAAAAAAAAAAAAAAAAAAAAAAAAAAAAAAAAAAAAAAAAAAAAAAAAAAAAAAA"
    "AAAAAAAAAAAAAAAAAAAAAAAAAAAAAAAAAAAAAAAAAAAAAAAAAAAAAAAAAAADCUBEABMFIwNIFAEB4hBIBgcQlMCI"
    "CDCkYoAgKAIZawAAAAAAAAAAAAAAAAAAAAAAAAAAAAAAAAAAAAAAAAAAAAAAAAAAAAAAAAAAAAAAAAAAAAAAAAAA"
    "AAAAAAAAAAAAAAAAAAAAAAAAAAAAAAAAAAAAAAAAAAAAAAAAAAAAAIoAAAgKAUAIA2ACAQYEACaAIABACIlkCIAm"
    "IEAAAYQCAAAAAAAAAAAAAAAAAAAAAAAAAAAAAAAAAAAAAAAAAACKhfVjRv0Wj3OGlM+4SlPJoSR5ywVB+iZCpABY"
    "jC+n4gAAAAAAAAAAAAAAAAAAAAAAAAAAAAAAAAAAAAAAAAAAAAEBAlBBAAPaowCAMFABFGMt0AMAAEBmEEFA0gQj"
    "dBA="
)


def _mask_np() -> np.ndarray:
    """[B, PQ] bool dropout mask (True = patch replaced by embed)."""
    bits = np.unpackbits(np.frombuffer(base64.b64decode(_MASK_B64), np.uint8))
    return bits[: B * PQ].reshape(B, PQ).astype(bool)


_CACHE: dict = {}

# The BassKernelResults of the most recent kernel() call (exec_time_ns is
# populated when BASS_TRACE=1 is set in the environment).
LAST_RESULTS = None


def _build_nc():
    import concourse.bacc as bacc
    import concourse.mybir as mybir
    from concourse.tile import TileContext

    f32 = mybir.dt.float32
    # Bacc (not raw Bass): its finalize() splits multi-wait instructions into
    # the 1-wait-per-instruction form TRN2 codegen requires.
    nc = bacc.Bacc(None)
    x = nc.declare_dram_parameter("x", [BPC, H, C, W], f32, isOutput=False)
    e = nc.declare_dram_parameter("e", [H, C, W], f32, isOutput=False)
    m = nc.declare_dram_parameter("m", [H, BPC, W], mybir.dt.uint8, isOutput=False)
    y = nc.declare_dram_parameter("y", [BPC, H, C, W], f32, isOutput=True)

    with TileContext(nc) as tc:
        with (
            tc.tile_pool(name="const", bufs=1) as cpool,
            tc.tile_pool(name="work", bufs=WORK_BUFS) as pool,
        ):
            et = cpool.tile([H, C, W], f32)
            nc.sync.dma_start(out=et[:], in_=e[:])
            mt = cpool.tile([H, BPC, W], mybir.dt.uint8)
            nc.sync.dma_start(out=mt[:], in_=m[:])
            # Touch the constants on DVE so its vector clock observes both
            # DMA semaphores here; otherwise the first copy_predicated needs
            # 3 sync waits, which overflows the TT encoding's wait slots.
            warm = cpool.tile([H, 2], f32)
            nc.vector.tensor_copy(out=warm[:, 0:1], in_=et[:, 0, 0:1])
            nc.vector.tensor_copy(out=warm[:, 1:2], in_=mt[:, 0, 0:1])
            # chunk schedule: (image, c_start, c_len). Uniform C//CHUNKS
            # splits, except the last image tapers off in smaller chunks so
            # the serial tail (last load -> predicate -> last store) shrinks.
            CH = C // CHUNKS
            sched = []
            for b in range(BPC - 1):
                for k in range(CHUNKS):
                    sched.append((b, k * CH, CH))
            for k in range(CHUNKS - 1):
                sched.append((BPC - 1, k * CH, CH))
            cs = (CHUNKS - 1) * CH
            for step in (CH // 2, CH // 4, CH // 8, CH // 8):
                sched.append((BPC - 1, cs, step))
                cs += step
            assert cs == C
            for b, c0, cl in sched:
                t = pool.tile([H, CH, W], f32, tag="t")
                nc.sync.dma_start(out=t[:, :cl, :], in_=x[b, :, c0 : c0 + cl, :])
                mask_ap = mt[:, b : b + 1, :].broadcast_to([H, cl, W])
                nc.vector.copy_predicated(t[:, :cl, :], mask_ap, et[:, c0 : c0 + cl, :])
                nc.scalar.dma_start(out=y[b, :, c0 : c0 + cl, :], in_=t[:, :cl, :])
    # run the Bacc passes (register allocation, wait splitting) now —
    # run_bass_via_pjrt serializes the module without calling finalize().
    nc.finalize()
    return nc


def _prep_const_inputs():
    """embed-image [h, c, w] and per-core expanded masks [h, b, w]."""
    mask = _mask_np()
    m_img = mask.reshape(B, 16, 16)
    m_exp = np.repeat(np.repeat(m_img, 8, axis=1), 8, axis=2)  # [B, H, W] bool
    per_core_m = []
    for ci in range(N_CORES):
        mm = m_exp[ci * BPC : (ci + 1) * BPC].transpose(1, 0, 2)  # [H, BPC, W]
        per_core_m.append(np.ascontiguousarray(mm, dtype=np.uint8))
    return per_core_m


def kernel(batch: np.ndarray, embed: np.ndarray) -> np.ndarray:
    from concourse.bass_utils import run_bass_kernel_spmd

    global LAST_RESULTS
    batch = np.asarray(batch, dtype=np.float32)
    embed = np.asarray(embed, dtype=np.float32)

    if "nc" not in _CACHE:
        _CACHE["nc"] = _build_nc()
        _CACHE["masks"] = _prep_const_inputs()
    nc = _CACHE["nc"]
    per_core_m = _CACHE["masks"]

    # embed [PQ, C, 8, 8] -> embed-image [h, c, w]
    e_hcw = embed.reshape(16, 16, C, 8, 8).transpose(0, 3, 2, 1, 4).reshape(H, C, W)
    e_hcw = np.ascontiguousarray(e_hcw, dtype=np.float32)

    in_maps = []
    for ci in range(N_CORES):
        xb = batch[ci * BPC : (ci + 1) * BPC]  # [BPC, C, H, W]
        xt = np.ascontiguousarray(xb.transpose(0, 2, 1, 3))  # [BPC, H, C, W]
        in_maps.append({"x": xt, "e": e_hcw, "m": per_core_m[ci]})

    res = run_bass_kernel_spmd(nc, in_maps, core_ids=list(range(N_CORES)))
    LAST_RESULTS = res

    out = np.empty((B, C, H, W), dtype=np.float32)
    for ci in range(N_CORES):
        yt = res.results[ci]["y"]  # [BPC, H, C, W]
        out[ci * BPC : (ci + 1) * BPC] = yt.transpose(0, 2, 1, 3)
    return out
